# revision 11
# baseline (speedup 1.0000x reference)
"""DAGNN (10-hop propagation + sigmoid gating) Bass kernel for 8 trn2 NeuronCores.

Strategy (1D node partition, SPMD-uniform schedule):
  - Host assigns nodes to (core, window, slot) with degree balancing so every
    core runs an identical instruction stream (one NEFF, 8 cores).
  - Node features live in a Shared DRAM replica as 256B rows: cols 0:64 hold
    bf16 values, cols 64:128 are never read (gather elem_size must be a
    multiple of 256B).
  - Per hop: dma_gather pulls per-edge rows (4 supers = 16 windows merged per
    gather instruction to amortize gpsimd descriptor-gen overhead); PE
    computes the segment-sum via one-hot indicator matmuls accumulating in
    fp32 PSUM. Indicators are host-precomputed fp8 tables streamed from DRAM
    (hop-invariant), so the Vector engine does no indicator work. The Scalar
    engine drains PSUM with deg^-1 scaling straight to bf16. A split
    AllGather (windows 0:48 early, 48:112 late) rebuilds the replica while
    the second half of the hop still computes.
  - Gating is fused into the hop loop: z_k = sum_f g_k*s, sig_k =
    sigmoid(z_k*sqrt(deg)), acc += sig_k * g_k — no final reload pass.
"""

import sys

sys.path.insert(0, "/opt/trn_rl_repo")

import numpy as np
import ml_dtypes

BF16 = ml_dtypes.bfloat16
FP8 = ml_dtypes.float8_e4m3fn


def _config(n_nodes, k_hops, n_cores, w_per_core, w_per_super, t_per_bucket):
    g = globals()
    g["N_NODES"] = n_nodes
    g["D"] = 64
    g["K_HOPS"] = k_hops
    g["N_CORES"] = n_cores
    g["WIN"] = 128
    g["W_PER_CORE"] = w_per_core
    assert w_per_core * n_cores * 128 >= n_nodes
    g["ROWS_PC"] = w_per_core * 128
    g["REP_ROWS"] = n_cores * g["ROWS_PC"]
    g["N_SRC_WIN"] = 4
    assert g["REP_ROWS"] % 4 == 0
    g["SRC_WIN"] = g["REP_ROWS"] // 4
    assert g["SRC_WIN"] <= 32768
    g["W_PER_SUPER"] = w_per_super
    g["SUPERS_PER_GRP"] = 4
    g["W_PER_GRP"] = w_per_super * g["SUPERS_PER_GRP"]
    assert w_per_core % g["W_PER_GRP"] == 0
    g["SUPERS"] = w_per_core // w_per_super
    g["GROUPS"] = w_per_core // g["W_PER_GRP"]
    g["T_PER_BUCKET"] = t_per_bucket
    g["SLOTS_PER_WS"] = t_per_bucket * 128
    # per (group, src-window) gather block
    g["GRP_SLOTS"] = g["W_PER_GRP"] * g["SLOTS_PER_WS"]
    g["GRP_TILES"] = g["GRP_SLOTS"] // 128
    g["SLOTS_TOTAL"] = w_per_core * 4 * g["SLOTS_PER_WS"]
    g["TILES_TOTAL"] = g["SLOTS_TOTAL"] // 128


_config(100000, 10, 8, 112, 4, 3)
WSPLIT = 56  # windows [0:56) AllGather early (tensor A), [56:112) late (B)
ROWS_A = WSPLIT * 128  # per-core rows in piece A
ROWS_B = (W_PER_CORE - WSPLIT) * 128


# ----------------------------------------------------------------------------
# Host preprocessing
# ----------------------------------------------------------------------------
def _balance_assign(deg_s_fn, tot):
    """Assign nodes to global windows (N_CORES*W_PER_CORE, cap 128 each) so
    that every (window, src-window) edge count stays <= SLOTS_PER_WS."""
    import heapq

    n = tot.shape[0]
    n_windows = N_CORES * W_PER_CORE
    order = np.argsort(-tot, kind="stable")
    heap = [(0, w) for w in range(n_windows)]
    heapq.heapify(heap)
    win_of = np.empty(n, dtype=np.int32)
    win_fill = np.zeros(n_windows, dtype=np.int32)
    for v in order:
        while True:
            load, w = heapq.heappop(heap)
            if win_fill[w] < WIN:
                break
        win_of[v] = w
        win_fill[w] += 1
        if win_fill[w] < WIN:
            heapq.heappush(heap, (load + int(tot[v]), w))

    rng = np.random.default_rng(12345)
    cap = SLOTS_PER_WS
    for round_i in range(12):
        pos_of = np.zeros(n, dtype=np.int32)
        ordv = np.lexsort((np.arange(n), win_of))
        posctr = np.zeros(n_windows, dtype=np.int32)
        for v in ordv:
            pos_of[v] = posctr[win_of[v]]
            posctr[win_of[v]] += 1
        core_of = (win_of // W_PER_CORE).astype(np.int32)
        wloc_of = (win_of % W_PER_CORE).astype(np.int32)
        deg_s = deg_s_fn(core_of, wloc_of, pos_of)  # [n, 4]
        loads = np.zeros((n_windows, N_SRC_WIN), dtype=np.int64)
        np.add.at(loads, win_of, deg_s)
        over = np.flatnonzero((loads > cap).any(axis=1))
        if len(over) == 0:
            return core_of, wloc_of, pos_of
        for w in over:
            s_bad = int(np.argmax(loads[w]))
            excess = int(loads[w, s_bad] - cap)
            members = np.flatnonzero(win_of == w)
            mdeg = deg_s[members, s_bad]
            for v in members[np.argsort(-mdeg)]:
                if excess <= 0:
                    break
                cands = rng.integers(0, n_windows, 64)
                best, bestval = -1, None
                for cw in cands:
                    if cw == w or posctr[cw] >= WIN:
                        continue
                    val = int((loads[cw] + deg_s[v]).max())
                    if val <= cap - 8 and (bestval is None or val < bestval):
                        best, bestval = int(cw), val
                if best < 0:
                    continue
                loads[w] -= deg_s[v]
                loads[best] += deg_s[v]
                win_of[v] = best
                posctr[w] -= 1
                posctr[best] += 1
                excess -= int(deg_s[v, s_bad])
    raise RuntimeError("balance repair failed to converge")


def _preprocess(feats, s, src, dst):
    src = np.asarray(src, dtype=np.int64)
    dst = np.asarray(dst, dtype=np.int64)
    n = N_NODES
    deg = np.bincount(dst, minlength=n).astype(np.float64)
    n2 = (1.0 / deg).astype(np.float32)
    norm = (deg ** -0.5).astype(np.float32)
    sqrtdeg = np.sqrt(deg).astype(np.float32)

    # ---- peel one self-loop per node (handled via identity matmul) ----
    loop_mask = src == dst
    loop_idx = np.flatnonzero(loop_mask)
    uniq_nodes, first_pos = np.unique(dst[loop_idx], return_index=True)
    if len(uniq_nodes) != n:
        raise RuntimeError("not every node has a self-loop; identity fold invalid")
    drop = np.zeros(len(src), dtype=bool)
    drop[loop_idx[first_pos]] = True
    src = src[~drop]
    dst = dst[~drop]

    # ---- node assignment (core, window, pos) ----
    deg_r = np.bincount(dst, minlength=n).astype(np.int64)

    def split_row(core_of, wloc_of, pos_of):
        """(src window id 0..3, row within window) under the A/B split
        layout: piece A = windows [0:WSPLIT) w-major rows, piece B rest."""
        c = core_of.astype(np.int64)
        w = wloc_of.astype(np.int64)
        p = pos_of.astype(np.int64)
        in_b = w >= WSPLIT
        local = np.where(in_b, (w - WSPLIT) * 128 + p, w * 128 + p)
        row = c * ROWS_A + local  # ROWS_A == ROWS_B
        half = row // SRC_WIN
        s = np.where(in_b, 2 + half, half)
        return s, row - half * SRC_WIN

    def deg_s_fn(core_of, wloc_of, pos_of):
        es, _ = split_row(core_of[src], wloc_of[src], pos_of[src])
        out = np.zeros((n, N_SRC_WIN), dtype=np.int64)
        np.add.at(out, (dst, es), 1)
        return out

    core_of, wloc_of, pos_of = _balance_assign(deg_s_fn, deg_r)

    # ---- per-core edge bucketing (group = 4 supers merged per gather) ----
    e_core = core_of[dst]
    e_w = wloc_of[dst]
    e_key = pos_of[dst]
    e_s, e_gidx = split_row(core_of[src], wloc_of[src], pos_of[src])

    gidx_all = np.zeros((N_CORES, SLOTS_TOTAL), dtype=np.int16)
    keys_all = np.full((N_CORES, SLOTS_TOTAL), -1, dtype=np.int32)

    for c in range(N_CORES):
        m = e_core == c
        cw = e_w[m]
        cs = e_s[m]
        ckey = e_key[m]
        cg = e_gidx[m]
        ws = cw * N_SRC_WIN + cs
        order = np.argsort(ws, kind="stable")
        cw, cs, ckey, cg, ws = cw[order], cs[order], ckey[order], cg[order], ws[order]
        counts = np.bincount(ws, minlength=W_PER_CORE * N_SRC_WIN)
        if counts.max() > SLOTS_PER_WS:
            raise RuntimeError(f"bucket overflow: {counts.max()} > {SLOTS_PER_WS}")
        # slot layout: (g, s) block of GRP_SLOTS; within it
        # sl (super in group) * 4*SLOTS_PER_WS + wi*SLOTS_PER_WS + runpos
        w_arr = np.arange(W_PER_CORE * N_SRC_WIN) // N_SRC_WIN
        s_arr = np.arange(W_PER_CORE * N_SRC_WIN) % N_SRC_WIN
        g_arr = w_arr // W_PER_GRP
        sl_arr = (w_arr % W_PER_GRP) // W_PER_SUPER
        wi_arr = w_arr % W_PER_SUPER
        starts = (
            (g_arr * N_SRC_WIN + s_arr) * GRP_SLOTS
            + sl_arr * (W_PER_SUPER * SLOTS_PER_WS)
            + wi_arr * SLOTS_PER_WS
        )
        runpos = np.arange(len(ws)) - np.repeat(
            np.concatenate([[0], np.cumsum(counts)[:-1]]), counts
        )
        slots = starts[ws] + runpos
        gidx_all[c, slots] = cg.astype(np.int16)
        keys_all[c, slots] = ckey.astype(np.int32)

    # ---- initial replica g0 = norm * feats in bf16, 256B rows, w-major ----
    g0 = (feats.astype(np.float32) * norm[:, None]).astype(BF16)
    g0_rep = np.zeros((REP_ROWS, 2 * D), dtype=BF16)
    own_row = (
        core_of.astype(np.int64) * ROWS_PC
        + wloc_of.astype(np.int64) * 128
        + pos_of.astype(np.int64)
    )
    g0_rep[own_row, 0:D] = g0

    def pw_table(vec):
        out = np.zeros((N_CORES, WIN, W_PER_CORE), dtype=np.float32)
        out[core_of, pos_of, wloc_of] = vec
        return out

    n2_pw = pw_table(n2)
    n2_pw[n2_pw == 0] = 1.0
    sqd_pw = pw_table(sqrtdeg)

    gidx_wrapped = np.ascontiguousarray(
        np.tile(
            gidx_all.reshape(N_CORES, SLOTS_TOTAL // 16, 16).transpose(0, 2, 1),
            (1, 8, 1),
        )
    )
    # indicator one-hot tables, fp8: ind[p, tile, f] = (key[slot=tile*128+p] == f)
    keys_pt = keys_all.reshape(N_CORES, TILES_TOTAL, 128).transpose(0, 2, 1)
    fvals = np.arange(128, dtype=np.int32)
    ind_all = (keys_pt[:, :, :, None] == fvals).astype(FP8)  # [C,128,TILES,128]
    ind_all = ind_all.reshape(N_CORES, 128, TILES_TOTAL * 128)

    ident8 = np.ascontiguousarray(np.eye(128, dtype=np.float32).astype(FP8))
    s_bcast = np.broadcast_to(
        np.asarray(s, dtype=np.float32).reshape(1, D), (128, D)
    ).astype(BF16)
    s_bcast = np.ascontiguousarray(s_bcast)

    in_maps = []
    for c in range(N_CORES):
        in_maps.append(
            {
                "g0_own": np.ascontiguousarray(
                    g0_rep[c * ROWS_PC : (c + 1) * ROWS_PC]
                ),
                "gidx": gidx_wrapped[c],
                "indt": np.ascontiguousarray(ind_all[c]),
                "n2_pw": np.ascontiguousarray(n2_pw[c]),
                "sqd_pw": np.ascontiguousarray(sqd_pw[c]),
                "s_bcast": s_bcast,
                "ident8": ident8,
            }
        )
    meta = {
        "core_of": core_of,
        "wloc_of": wloc_of,
        "pos_of": pos_of,
    }
    return in_maps, meta


# ----------------------------------------------------------------------------
# Bass kernel builder (identical program for all cores)
# ----------------------------------------------------------------------------
def _build():
    import concourse.bacc as bacc
    import concourse.mybir as mybir
    from concourse.tile import TileContext

    fp32 = mybir.dt.float32
    bf16 = mybir.dt.bfloat16
    fp8 = mybir.dt.float8e4
    i16 = mybir.dt.int16

    nc = bacc.Bacc(None, target_bir_lowering=False, num_devices=N_CORES, num_swdge_queues=4)

    # I/O
    g0_own = nc.dram_tensor("g0_own", [ROWS_PC, 2 * D], bf16, kind="ExternalInput")
    gidx_in = nc.dram_tensor("gidx", [128, SLOTS_TOTAL // 16], i16, kind="ExternalInput")
    indt_in = nc.dram_tensor("indt", [128, TILES_TOTAL * 128], fp8, kind="ExternalInput")
    n2_in = nc.dram_tensor("n2_pw", [128, W_PER_CORE], fp32, kind="ExternalInput")
    sqd_in = nc.dram_tensor("sqd_pw", [128, W_PER_CORE], fp32, kind="ExternalInput")
    s_in = nc.dram_tensor("s_bcast", [128, D], bf16, kind="ExternalInput")
    ident_in = nc.dram_tensor("ident8", [128, 128], fp8, kind="ExternalInput")
    out_pm = nc.dram_tensor("out_pm", [ROWS_PC, D], fp32, kind="ExternalOutput")

    cc_in_a = [
        nc.dram_tensor(f"cc_in_a{k}", [ROWS_A, 2 * D], bf16) for k in range(K_HOPS)
    ]
    cc_in_b = [
        nc.dram_tensor(f"cc_in_b{k}", [ROWS_B, 2 * D], bf16) for k in range(K_HOPS)
    ]
    cc_out_a = [
        nc.dram_tensor(
            f"cc_out_a{k}", [N_CORES * ROWS_A, 2 * D], bf16, addr_space="Shared"
        )
        for k in range(K_HOPS)
    ]
    cc_out_b = [
        nc.dram_tensor(
            f"cc_out_b{k}", [N_CORES * ROWS_B, 2 * D], bf16, addr_space="Shared"
        )
        for k in range(K_HOPS)
    ]
    groups = [list(range(N_CORES))]

    with TileContext(nc) as tc:
        with tc.tile_pool(name="const", bufs=1) as const_pool:
            # ---- load static tables ----
            gidx_sb = const_pool.tile([128, SLOTS_TOTAL // 16], i16, tag="gidx")
            nc.sync.dma_start(out=gidx_sb[:, :], in_=gidx_in[:, :])
            n2_sb = const_pool.tile([128, W_PER_CORE], fp32, tag="n2")
            nc.sync.dma_start(out=n2_sb[:, :], in_=n2_in[:, :])
            sqd_sb = const_pool.tile([128, W_PER_CORE], fp32, tag="sqd")
            nc.sync.dma_start(out=sqd_sb[:, :], in_=sqd_in[:, :])
            s_sb = const_pool.tile([128, D], bf16, tag="svec")
            nc.sync.dma_start(out=s_sb[:, :], in_=s_in[:, :])
            ident_sb = const_pool.tile([128, 128], fp8, tag="ident")
            nc.sync.dma_start(out=ident_sb[:, :], in_=ident_in[:, :])

            # gating state
            zt_sb = const_pool.tile([128, W_PER_CORE], fp32, tag="zt")
            sig_sb = const_pool.tile([128, W_PER_CORE], fp32, tag="sig")
            acc_sb = const_pool.tile([128, W_PER_CORE, D], fp32, tag="acc")
            zscr_sb = const_pool.tile([128, D], bf16, tag="zscr")
            nc.vector.memset(acc_sb[:, :, :], 0.0)

            def gate_accum(staged):
                for w in range(W_PER_CORE):
                    nc.vector.tensor_tensor(
                        zscr_sb[:, :],
                        staged[:, w, :],
                        s_sb[:, :],
                        mybir.AluOpType.mult,
                    )
                    nc.vector.tensor_reduce(
                        zt_sb[:, w : w + 1],
                        zscr_sb[:, :],
                        mybir.AxisListType.X,
                        mybir.AluOpType.add,
                    )
                nc.vector.tensor_tensor(
                    zt_sb[:, :], zt_sb[:, :], sqd_sb[:, :], mybir.AluOpType.mult
                )
                nc.scalar.activation(
                    sig_sb[:, :],
                    zt_sb[:, :],
                    mybir.ActivationFunctionType.Sigmoid,
                )
                for w in range(W_PER_CORE):
                    nc.vector.scalar_tensor_tensor(
                        acc_sb[:, w, :],
                        staged[:, w, :],
                        sig_sb[:, w : w + 1],
                        acc_sb[:, w, :],
                        mybir.AluOpType.mult,
                        mybir.AluOpType.add,
                    )

            with (
                tc.tile_pool(name="chunks", bufs=8) as chunk_pool,
                tc.tile_pool(name="inds", bufs=8) as ind_pool,
                tc.tile_pool(name="stage", bufs=2) as stage_pool,
                tc.tile_pool(name="psum", bufs=8, space="PSUM") as psum_pool,
            ):

                # ---- bootstrap ----
                nc.sync.dma_start(out=cc_in_a[0][:, :], in_=g0_own[0:ROWS_A, :])
                nc.sync.dma_start(out=cc_in_b[0][:, :], in_=g0_own[ROWS_A:, :])
                nc.gpsimd.collective_compute(
                    "AllGather",
                    mybir.AluOpType.bypass,
                    replica_groups=groups,
                    ins=[cc_in_a[0][:, :]],
                    outs=[cc_out_a[0][:, :]],
                )
                nc.gpsimd.collective_compute(
                    "AllGather",
                    mybir.AluOpType.bypass,
                    replica_groups=groups,
                    ins=[cc_in_b[0][:, :]],
                    outs=[cc_out_b[0][:, :]],
                )
                staged_prev = stage_pool.tile([128, W_PER_CORE, D], bf16, tag="staged")
                nc.sync.dma_start(
                    out=staged_prev[:, :, :],
                    in_=g0_own[:, 0:D].rearrange("(w p) f -> p w f", p=128),
                )
                gate_accum(staged_prev)

                def drain_super(sup, banks, staged):
                    for wi in range(W_PER_SUPER):
                        w = sup * W_PER_SUPER + wi
                        nc.scalar.activation(
                            staged[:, w, :],
                            banks[wi][:, :],
                            mybir.ActivationFunctionType.Copy,
                            scale=n2_sb[:, w : w + 1],
                        )

                def send_piece(hop, staged, w0, w1):
                    """DMA staged windows [w0:w1) to its cc_in piece, then
                    AllGather the whole (contiguous) piece tensor."""
                    if hop >= K_HOPS - 1:
                        return
                    t_in = cc_in_a[hop + 1] if w0 == 0 else cc_in_b[hop + 1]
                    t_out = cc_out_a[hop + 1] if w0 == 0 else cc_out_b[hop + 1]
                    nc.sync.dma_start(
                        out=t_in[:, 0:D].rearrange("(w p) f -> p w f", p=128),
                        in_=staged[:, w0:w1, :],
                    )
                    nc.gpsimd.collective_compute(
                        "AllGather",
                        mybir.AluOpType.bypass,
                        replica_groups=groups,
                        ins=[t_in[:, :]],
                        outs=[t_out[:, :]],
                    )

                SPLIT_SUP = WSPLIT // W_PER_SUPER  # drain of sup SPLIT_SUP-1 ends piece A

                for hop in range(K_HOPS):
                    staged = stage_pool.tile([128, W_PER_CORE, D], bf16, tag="staged")
                    pending = None
                    for grp in range(GROUPS):
                        chunks = []
                        for s in range(N_SRC_WIN):
                            ch = chunk_pool.tile(
                                [128, GRP_TILES, 2 * D], bf16, tag="chunk"
                            )
                            col0 = (grp * N_SRC_WIN + s) * (GRP_SLOTS // 16)
                            src_rep = cc_out_a[hop] if s < 2 else cc_out_b[hop]
                            row0 = (s % 2) * SRC_WIN
                            nc.gpsimd.dma_gather(
                                ch[:, :, :],
                                src_rep[row0 : row0 + SRC_WIN, :],
                                gidx_sb[:, col0 : col0 + GRP_SLOTS // 16],
                                GRP_SLOTS,
                                GRP_SLOTS,
                                2 * D,
                                single_packet=False,
                                queue_num=s,
                            )
                            chunks.append(ch)
                        for sl in range(SUPERS_PER_GRP):
                            sup = grp * SUPERS_PER_GRP + sl
                            banks = [
                                psum_pool.tile([128, D], fp32, tag="bank", name="bank")
                                for _ in range(W_PER_SUPER)
                            ]
                            for s in range(N_SRC_WIN):
                                wt = W_PER_SUPER * T_PER_BUCKET
                                tile0 = (grp * N_SRC_WIN + s) * GRP_TILES + sl * wt
                                indb = ind_pool.tile([128, wt, 128], fp8, tag="ind")
                                nc.sync.dma_start(
                                    out=indb[:, :, :],
                                    in_=indt_in[:, tile0 * 128 : (tile0 + wt) * 128],
                                )
                                for wi in range(W_PER_SUPER):
                                    w = sup * W_PER_SUPER + wi
                                    bank = banks[wi]
                                    for t in range(T_PER_BUCKET):
                                        j = sl * wt + wi * T_PER_BUCKET + t
                                        nc.tensor.matmul(
                                            bank[:, :],
                                            indb[:, wi * T_PER_BUCKET + t, :],
                                            chunks[s][:, j, 0:D],
                                            start=(s == 0 and t == 0),
                                            stop=False,
                                        )
                                    if s == N_SRC_WIN - 1:
                                        nc.tensor.matmul(
                                            bank[:, :],
                                            ident_sb[:, :],
                                            staged_prev[:, w, :],
                                            start=False,
                                            stop=True,
                                        )
                            if pending is not None:
                                drain_super(pending[0], pending[1], staged)
                                if pending[0] == SPLIT_SUP - 1:
                                    send_piece(hop, staged, 0, WSPLIT)
                            pending = (sup, banks)
                    drain_super(pending[0], pending[1], staged)
                    send_piece(hop, staged, WSPLIT, W_PER_CORE)
                    gate_accum(staged)
                    staged_prev = staged

            # ---- final: out = acc * sqrtdeg (in place), then store ----
            for w in range(W_PER_CORE):
                nc.vector.tensor_scalar_mul(
                    acc_sb[:, w, :], acc_sb[:, w, :], sqd_sb[:, w : w + 1]
                )
            nc.sync.dma_start(
                out=out_pm[:, :].rearrange("(p w) f -> p (w f)", p=128),
                in_=acc_sb[:, :, :],
            )

    nc.finalize()
    return nc


# ----------------------------------------------------------------------------
# Entry point
# ----------------------------------------------------------------------------
_CACHED = {}


def kernel(**inputs):
    feats = np.asarray(inputs["feats"], dtype=np.float32)
    s = np.asarray(inputs["s"], dtype=np.float32)
    src = np.asarray(inputs["src"])
    dst = np.asarray(inputs["dst"])

    in_maps, meta = _preprocess(feats, s, src, dst)

    from concourse.bass_utils import run_bass_kernel_spmd

    nc = _CACHED.get("nc")
    if nc is None:
        nc = _build()
        _CACHED["nc"] = nc

    res = run_bass_kernel_spmd(nc, in_maps, core_ids=list(range(N_CORES)))
    _CACHED["last_result"] = res
    out = np.zeros((N_NODES, D), dtype=np.float32)
    core_of, wloc_of, pos_of = meta["core_of"], meta["wloc_of"], meta["pos_of"]
    rows = pos_of.astype(np.int64) * W_PER_CORE + wloc_of.astype(np.int64)
    for c in range(N_CORES):
        m = core_of == c
        out[m] = res.results[c]["out_pm"][rows[m]]
    return out


if __name__ == "__main__":
    nc = _build()
    print("build ok")


# revision 17
# speedup vs baseline: 1.4661x; 1.4661x over previous
"""DAGNN (10-hop propagation + sigmoid gating) Bass kernel for 8 trn2 NeuronCores.

Strategy (1D node partition, SPMD-uniform schedule):
  - Host assigns nodes to (core, window, slot) with degree balancing so every
    core runs an identical instruction stream (one NEFF, 8 cores).
  - Node features live in a Shared DRAM replica as 256B rows: cols 0:64 hold
    bf16 values, cols 64:128 are never read (gather elem_size must be a
    multiple of 256B).
  - Per hop: dma_gather pulls per-edge rows (4 supers = 16 windows merged per
    gather instruction to amortize gpsimd descriptor-gen overhead); PE
    computes the segment-sum via one-hot indicator matmuls accumulating in
    fp32 PSUM. Indicators are host-precomputed fp8 tables streamed from DRAM
    (hop-invariant), so the Vector engine does no indicator work. The Scalar
    engine drains PSUM with deg^-1 scaling straight to bf16. A split
    AllGather (windows 0:48 early, 48:112 late) rebuilds the replica while
    the second half of the hop still computes.
  - Gating is fused into the hop loop: z_k = sum_f g_k*s, sig_k =
    sigmoid(z_k*sqrt(deg)), acc += sig_k * g_k — no final reload pass.
"""

import sys

sys.path.insert(0, "/opt/trn_rl_repo")

import numpy as np
import ml_dtypes

BF16 = ml_dtypes.bfloat16
FP8 = ml_dtypes.float8_e4m3fn


def _config(n_nodes, k_hops, n_cores, w_per_core, w_per_super, t_per_bucket):
    g = globals()
    g["N_NODES"] = n_nodes
    g["D"] = 64
    g["K_HOPS"] = k_hops
    g["N_CORES"] = n_cores
    g["WIN"] = 128
    g["W_PER_CORE"] = w_per_core
    assert w_per_core * n_cores * 128 >= n_nodes
    g["ROWS_PC"] = w_per_core * 128
    g["REP_ROWS"] = n_cores * g["ROWS_PC"]
    g["N_SRC_WIN"] = 4
    assert g["REP_ROWS"] % 4 == 0
    g["SRC_WIN"] = g["REP_ROWS"] // 4
    assert g["SRC_WIN"] <= 32768
    g["W_PER_SUPER"] = w_per_super
    g["SUPERS_PER_GRP"] = 2
    g["W_PER_GRP"] = w_per_super * g["SUPERS_PER_GRP"]
    assert w_per_core % g["W_PER_GRP"] == 0
    g["SUPERS"] = w_per_core // w_per_super
    g["GROUPS"] = w_per_core // g["W_PER_GRP"]
    g["T_PER_BUCKET"] = t_per_bucket
    g["SLOTS_PER_WS"] = t_per_bucket * 128
    # per (group, src-window) gather block
    g["GRP_SLOTS"] = g["W_PER_GRP"] * g["SLOTS_PER_WS"]
    g["GRP_TILES"] = g["GRP_SLOTS"] // 128
    g["SLOTS_TOTAL"] = w_per_core * 4 * g["SLOTS_PER_WS"]
    g["TILES_TOTAL"] = g["SLOTS_TOTAL"] // 128


_config(100000, 10, 8, 104, 4, 3)
WSPLIT = 52  # windows [0:52) AllGather early (tensor A), [52:104) late (B)
ROWS_A = WSPLIT * 128  # per-core rows in piece A
ROWS_B = (W_PER_CORE - WSPLIT) * 128
assert ROWS_A == ROWS_B and N_CORES * ROWS_A == 2 * SRC_WIN


# ----------------------------------------------------------------------------
# Host preprocessing
# ----------------------------------------------------------------------------
def _balance_assign(deg_s_fn, tot):
    """Assign nodes to global windows (N_CORES*W_PER_CORE, cap 128 each) so
    that every (window, src-window) edge count stays <= SLOTS_PER_WS."""
    import heapq

    n = tot.shape[0]
    n_windows = N_CORES * W_PER_CORE
    order = np.argsort(-tot, kind="stable")
    heap = [(0, w) for w in range(n_windows)]
    heapq.heapify(heap)
    win_of = np.empty(n, dtype=np.int32)
    win_fill = np.zeros(n_windows, dtype=np.int32)
    for v in order:
        while True:
            load, w = heapq.heappop(heap)
            if win_fill[w] < WIN:
                break
        win_of[v] = w
        win_fill[w] += 1
        if win_fill[w] < WIN:
            heapq.heappush(heap, (load + int(tot[v]), w))

    rng = np.random.default_rng(12345)
    cap = SLOTS_PER_WS
    for round_i in range(12):
        pos_of = np.zeros(n, dtype=np.int32)
        ordv = np.lexsort((np.arange(n), win_of))
        posctr = np.zeros(n_windows, dtype=np.int32)
        for v in ordv:
            pos_of[v] = posctr[win_of[v]]
            posctr[win_of[v]] += 1
        core_of = (win_of // W_PER_CORE).astype(np.int32)
        wloc_of = (win_of % W_PER_CORE).astype(np.int32)
        deg_s = deg_s_fn(core_of, wloc_of, pos_of)  # [n, 4]
        loads = np.zeros((n_windows, N_SRC_WIN), dtype=np.int64)
        np.add.at(loads, win_of, deg_s)
        over = np.flatnonzero((loads > cap).any(axis=1))
        if len(over) == 0:
            return core_of, wloc_of, pos_of
        for w in over:
            s_bad = int(np.argmax(loads[w]))
            excess = int(loads[w, s_bad] - cap)
            members = np.flatnonzero(win_of == w)
            mdeg = deg_s[members, s_bad]
            for v in members[np.argsort(-mdeg)]:
                if excess <= 0:
                    break
                cands = rng.integers(0, n_windows, 64)
                best, bestval = -1, None
                for cw in cands:
                    if cw == w or posctr[cw] >= WIN:
                        continue
                    val = int((loads[cw] + deg_s[v]).max())
                    if val <= cap - 8 and (bestval is None or val < bestval):
                        best, bestval = int(cw), val
                if best < 0:
                    continue
                loads[w] -= deg_s[v]
                loads[best] += deg_s[v]
                win_of[v] = best
                posctr[w] -= 1
                posctr[best] += 1
                excess -= int(deg_s[v, s_bad])
    raise RuntimeError("balance repair failed to converge")


def _preprocess(feats, s, src, dst):
    src = np.asarray(src, dtype=np.int64)
    dst = np.asarray(dst, dtype=np.int64)
    n = N_NODES
    deg = np.bincount(dst, minlength=n).astype(np.float64)
    n2 = (1.0 / deg).astype(np.float32)
    norm = (deg ** -0.5).astype(np.float32)
    sqrtdeg = np.sqrt(deg).astype(np.float32)

    # ---- peel one self-loop per node (handled via identity matmul) ----
    loop_mask = src == dst
    loop_idx = np.flatnonzero(loop_mask)
    uniq_nodes, first_pos = np.unique(dst[loop_idx], return_index=True)
    if len(uniq_nodes) != n:
        raise RuntimeError("not every node has a self-loop; identity fold invalid")
    drop = np.zeros(len(src), dtype=bool)
    drop[loop_idx[first_pos]] = True
    src = src[~drop]
    dst = dst[~drop]

    # ---- node assignment (core, window, pos) ----
    deg_r = np.bincount(dst, minlength=n).astype(np.int64)

    def split_row(core_of, wloc_of, pos_of):
        """(src window id 0..3, row within window) under the A/B split
        layout: piece A = windows [0:WSPLIT) w-major rows, piece B rest."""
        c = core_of.astype(np.int64)
        w = wloc_of.astype(np.int64)
        p = pos_of.astype(np.int64)
        in_b = w >= WSPLIT
        local = np.where(in_b, (w - WSPLIT) * 128 + p, w * 128 + p)
        row = c * ROWS_A + local  # ROWS_A == ROWS_B
        half = row // SRC_WIN
        s = np.where(in_b, 2 + half, half)
        return s, row - half * SRC_WIN

    def deg_s_fn(core_of, wloc_of, pos_of):
        es, _ = split_row(core_of[src], wloc_of[src], pos_of[src])
        out = np.zeros((n, N_SRC_WIN), dtype=np.int64)
        np.add.at(out, (dst, es), 1)
        return out

    core_of, wloc_of, pos_of = _balance_assign(deg_s_fn, deg_r)

    # ---- per-core edge bucketing (group = 4 supers merged per gather) ----
    e_core = core_of[dst]
    e_w = wloc_of[dst]
    e_key = pos_of[dst]
    e_s, e_gidx = split_row(core_of[src], wloc_of[src], pos_of[src])

    gidx_all = np.zeros((N_CORES, SLOTS_TOTAL), dtype=np.int16)
    keys_all = np.full((N_CORES, SLOTS_TOTAL), -1, dtype=np.int32)

    for c in range(N_CORES):
        m = e_core == c
        cw = e_w[m]
        cs = e_s[m]
        ckey = e_key[m]
        cg = e_gidx[m]
        ws = cw * N_SRC_WIN + cs
        order = np.argsort(ws, kind="stable")
        cw, cs, ckey, cg, ws = cw[order], cs[order], ckey[order], cg[order], ws[order]
        counts = np.bincount(ws, minlength=W_PER_CORE * N_SRC_WIN)
        if counts.max() > SLOTS_PER_WS:
            raise RuntimeError(f"bucket overflow: {counts.max()} > {SLOTS_PER_WS}")
        # slot layout: (g, s) block of GRP_SLOTS; within it
        # sl (super in group) * 4*SLOTS_PER_WS + wi*SLOTS_PER_WS + runpos
        w_arr = np.arange(W_PER_CORE * N_SRC_WIN) // N_SRC_WIN
        s_arr = np.arange(W_PER_CORE * N_SRC_WIN) % N_SRC_WIN
        g_arr = w_arr // W_PER_GRP
        sl_arr = (w_arr % W_PER_GRP) // W_PER_SUPER
        wi_arr = w_arr % W_PER_SUPER
        starts = (
            (g_arr * N_SRC_WIN + s_arr) * GRP_SLOTS
            + sl_arr * (W_PER_SUPER * SLOTS_PER_WS)
            + wi_arr * SLOTS_PER_WS
        )
        runpos = np.arange(len(ws)) - np.repeat(
            np.concatenate([[0], np.cumsum(counts)[:-1]]), counts
        )
        slots = starts[ws] + runpos
        gidx_all[c, slots] = cg.astype(np.int16)
        keys_all[c, slots] = ckey.astype(np.int32)

    # ---- initial replica g0 = norm * feats in bf16, 256B rows, w-major ----
    g0 = (feats.astype(np.float32) * norm[:, None]).astype(BF16)
    g0_rep = np.zeros((REP_ROWS, 2 * D), dtype=BF16)
    own_row = (
        core_of.astype(np.int64) * ROWS_PC
        + wloc_of.astype(np.int64) * 128
        + pos_of.astype(np.int64)
    )
    g0_rep[own_row, 0:D] = g0

    def pw_table(vec):
        out = np.zeros((N_CORES, WIN, W_PER_CORE), dtype=np.float32)
        out[core_of, pos_of, wloc_of] = vec
        return out

    n2_pw = pw_table(n2)
    n2_pw[n2_pw == 0] = 1.0
    sqd_pw = pw_table(sqrtdeg)

    gidx_wrapped = np.ascontiguousarray(
        np.tile(
            gidx_all.reshape(N_CORES, SLOTS_TOTAL // 16, 16).transpose(0, 2, 1),
            (1, 8, 1),
        )
    )
    # indicator one-hot tables, fp8: ind[p, tile, f] = (key[slot=tile*128+p] == f)
    keys_pt = keys_all.reshape(N_CORES, TILES_TOTAL, 128).transpose(0, 2, 1)
    fvals = np.arange(128, dtype=np.int32)
    ind_all = (keys_pt[:, :, :, None] == fvals).astype(FP8)  # [C,128,TILES,128]
    ind_all = ind_all.reshape(N_CORES, 128, TILES_TOTAL * 128)

    ident8 = np.ascontiguousarray(np.eye(128, dtype=np.float32).astype(FP8))
    s_bcast = np.broadcast_to(
        np.asarray(s, dtype=np.float32).reshape(1, D), (128, D)
    ).astype(BF16)
    s_bcast = np.ascontiguousarray(s_bcast)

    in_maps = []
    for c in range(N_CORES):
        in_maps.append(
            {
                "g0_own": np.ascontiguousarray(
                    g0_rep[c * ROWS_PC : (c + 1) * ROWS_PC]
                ),
                "gidx": gidx_wrapped[c],
                "indt": np.ascontiguousarray(ind_all[c]),
                "n2_pw": np.ascontiguousarray(n2_pw[c]),
                "sqd_pw": np.ascontiguousarray(sqd_pw[c]),
                "s_bcast": s_bcast,
                "ident8": ident8,
            }
        )
    meta = {
        "core_of": core_of,
        "wloc_of": wloc_of,
        "pos_of": pos_of,
    }
    return in_maps, meta


# ----------------------------------------------------------------------------
# Bass kernel builder (identical program for all cores)
# ----------------------------------------------------------------------------
def _build():
    import concourse.bacc as bacc
    import concourse.mybir as mybir
    from concourse.tile import TileContext

    fp32 = mybir.dt.float32
    bf16 = mybir.dt.bfloat16
    fp8 = mybir.dt.float8e4
    i16 = mybir.dt.int16

    nc = bacc.Bacc(None, target_bir_lowering=False, num_devices=N_CORES, num_swdge_queues=4)

    # I/O
    g0_own = nc.dram_tensor("g0_own", [ROWS_PC, 2 * D], bf16, kind="ExternalInput")
    gidx_in = nc.dram_tensor("gidx", [128, SLOTS_TOTAL // 16], i16, kind="ExternalInput")
    indt_in = nc.dram_tensor("indt", [128, TILES_TOTAL * 128], fp8, kind="ExternalInput")
    n2_in = nc.dram_tensor("n2_pw", [128, W_PER_CORE], fp32, kind="ExternalInput")
    sqd_in = nc.dram_tensor("sqd_pw", [128, W_PER_CORE], fp32, kind="ExternalInput")
    s_in = nc.dram_tensor("s_bcast", [128, D], bf16, kind="ExternalInput")
    ident_in = nc.dram_tensor("ident8", [128, 128], fp8, kind="ExternalInput")
    out_pm = nc.dram_tensor("out_pm", [ROWS_PC, D], fp32, kind="ExternalOutput")

    cc_in_a = [
        nc.dram_tensor(f"cc_in_a{k}", [ROWS_A, 2 * D], bf16) for k in range(K_HOPS)
    ]
    cc_in_b = [
        nc.dram_tensor(f"cc_in_b{k}", [ROWS_B, 2 * D], bf16) for k in range(K_HOPS)
    ]
    cc_out_a = [
        nc.dram_tensor(
            f"cc_out_a{k}", [N_CORES * ROWS_A, 2 * D], bf16, addr_space="Shared"
        )
        for k in range(K_HOPS)
    ]
    cc_out_b = [
        nc.dram_tensor(
            f"cc_out_b{k}", [N_CORES * ROWS_B, 2 * D], bf16, addr_space="Shared"
        )
        for k in range(K_HOPS)
    ]
    groups = [list(range(N_CORES))]

    with TileContext(nc) as tc:
        with tc.tile_pool(name="const", bufs=1) as const_pool:
            # ---- load static tables ----
            gidx_sb = const_pool.tile([128, SLOTS_TOTAL // 16], i16, tag="gidx")
            nc.sync.dma_start(out=gidx_sb[:, :], in_=gidx_in[:, :])
            n2_sb = const_pool.tile([128, W_PER_CORE], fp32, tag="n2")
            nc.sync.dma_start(out=n2_sb[:, :], in_=n2_in[:, :])
            sqd_sb = const_pool.tile([128, W_PER_CORE], fp32, tag="sqd")
            nc.sync.dma_start(out=sqd_sb[:, :], in_=sqd_in[:, :])
            s_sb = const_pool.tile([128, D], bf16, tag="svec")
            nc.sync.dma_start(out=s_sb[:, :], in_=s_in[:, :])
            ident_sb = const_pool.tile([128, 128], fp8, tag="ident")
            nc.sync.dma_start(out=ident_sb[:, :], in_=ident_in[:, :])

            # gating state
            zt_sb = const_pool.tile([128, W_PER_CORE], fp32, tag="zt")
            sig_sb = const_pool.tile([128, W_PER_CORE], fp32, tag="sig")
            acc_sb = const_pool.tile([128, W_PER_CORE, D], fp32, tag="acc")
            zuf_sb = const_pool.tile([128, W_PER_CORE, D], bf16, tag="zuf")
            nc.vector.memset(acc_sb[:, :, :], 0.0)

            def gate_accum(staged):
                nc.vector.tensor_tensor(
                    zuf_sb[:, :, :],
                    staged[:, :, :],
                    s_sb[:, :]
                    .rearrange("p (one f) -> p one f", one=1)
                    .broadcast_to((128, W_PER_CORE, D)),
                    mybir.AluOpType.mult,
                )
                nc.vector.tensor_reduce(
                    zt_sb[:, :],
                    zuf_sb[:, :, :],
                    mybir.AxisListType.X,
                    mybir.AluOpType.add,
                )
                nc.vector.tensor_tensor(
                    zt_sb[:, :], zt_sb[:, :], sqd_sb[:, :], mybir.AluOpType.mult
                )
                nc.scalar.activation(
                    sig_sb[:, :],
                    zt_sb[:, :],
                    mybir.ActivationFunctionType.Sigmoid,
                )
                for w in range(W_PER_CORE):
                    nc.vector.scalar_tensor_tensor(
                        acc_sb[:, w, :],
                        staged[:, w, :],
                        sig_sb[:, w : w + 1],
                        acc_sb[:, w, :],
                        mybir.AluOpType.mult,
                        mybir.AluOpType.add,
                    )

            with (
                tc.tile_pool(name="chunks", bufs=8) as chunk_pool,
                tc.tile_pool(name="inds", bufs=2) as ind_pool,
                tc.tile_pool(name="stage", bufs=2) as stage_pool,
                tc.tile_pool(name="psum", bufs=8, space="PSUM") as psum_pool,
            ):

                # ---- bootstrap ----
                nc.sync.dma_start(out=cc_in_a[0][:, :], in_=g0_own[0:ROWS_A, :])
                nc.sync.dma_start(out=cc_in_b[0][:, :], in_=g0_own[ROWS_A:, :])
                nc.gpsimd.collective_compute(
                    "AllGather",
                    mybir.AluOpType.bypass,
                    replica_groups=groups,
                    ins=[cc_in_a[0][:, :]],
                    outs=[cc_out_a[0][:, :]],
                )
                nc.gpsimd.collective_compute(
                    "AllGather",
                    mybir.AluOpType.bypass,
                    replica_groups=groups,
                    ins=[cc_in_b[0][:, :]],
                    outs=[cc_out_b[0][:, :]],
                )
                staged_prev = stage_pool.tile([128, W_PER_CORE, D], bf16, tag="staged")
                nc.sync.dma_start(
                    out=staged_prev[:, :, :],
                    in_=g0_own[:, 0:D].rearrange("(w p) f -> p w f", p=128),
                )
                gate_accum(staged_prev)

                def drain_super(sup, banks, staged):
                    for wi in range(W_PER_SUPER):
                        w = sup * W_PER_SUPER + wi
                        nc.scalar.activation(
                            staged[:, w, :],
                            banks[wi][:, :],
                            mybir.ActivationFunctionType.Copy,
                            scale=n2_sb[:, w : w + 1],
                        )

                def send_piece(hop, staged, w0, w1):
                    """DMA staged windows [w0:w1) to its cc_in piece, then
                    AllGather the whole (contiguous) piece tensor."""
                    if hop >= K_HOPS - 1:
                        return
                    t_in = cc_in_a[hop + 1] if w0 == 0 else cc_in_b[hop + 1]
                    t_out = cc_out_a[hop + 1] if w0 == 0 else cc_out_b[hop + 1]
                    nc.sync.dma_start(
                        out=t_in[:, 0:D].rearrange("(w p) f -> p w f", p=128),
                        in_=staged[:, w0:w1, :],
                    )
                    nc.gpsimd.collective_compute(
                        "AllGather",
                        mybir.AluOpType.bypass,
                        replica_groups=groups,
                        ins=[t_in[:, :]],
                        outs=[t_out[:, :]],
                    )

                SPLIT_SUP = WSPLIT // W_PER_SUPER  # drain of sup SPLIT_SUP-1 ends piece A

                for hop in range(K_HOPS):
                    staged = stage_pool.tile([128, W_PER_CORE, D], bf16, tag="staged")
                    pending = None
                    for grp in range(GROUPS):
                        # all 4*GRP_TILES indicator tiles of this group in one DMA
                        itile0 = grp * N_SRC_WIN * GRP_TILES
                        indg = ind_pool.tile(
                            [128, N_SRC_WIN * GRP_TILES, 128], fp8, tag="ind"
                        )
                        nc.sync.dma_start(
                            out=indg[:, :, :],
                            in_=indt_in[
                                :,
                                itile0 * 128 : (itile0 + N_SRC_WIN * GRP_TILES) * 128,
                            ],
                        )
                        chunks = []
                        for s in range(N_SRC_WIN):
                            ch = chunk_pool.tile(
                                [128, GRP_TILES, 2 * D], bf16, tag="chunk"
                            )
                            col0 = (grp * N_SRC_WIN + s) * (GRP_SLOTS // 16)
                            src_rep = cc_out_a[hop] if s < 2 else cc_out_b[hop]
                            row0 = (s % 2) * SRC_WIN
                            nc.gpsimd.dma_gather(
                                ch[:, :, :],
                                src_rep[row0 : row0 + SRC_WIN, :],
                                gidx_sb[:, col0 : col0 + GRP_SLOTS // 16],
                                GRP_SLOTS,
                                GRP_SLOTS,
                                2 * D,
                                single_packet=False,
                                queue_num=s,
                            )
                            chunks.append(ch)
                        wt = W_PER_SUPER * T_PER_BUCKET
                        for sl in range(SUPERS_PER_GRP):
                            sup = grp * SUPERS_PER_GRP + sl
                            banks = [
                                psum_pool.tile([128, D], fp32, tag="bank", name="bank")
                                for _ in range(W_PER_SUPER)
                            ]
                            for s in range(N_SRC_WIN):
                                for wi in range(W_PER_SUPER):
                                    w = sup * W_PER_SUPER + wi
                                    bank = banks[wi]
                                    for t in range(T_PER_BUCKET):
                                        j = sl * wt + wi * T_PER_BUCKET + t
                                        nc.tensor.matmul(
                                            bank[:, :],
                                            indg[:, s * GRP_TILES + j, :],
                                            chunks[s][:, j, 0:D],
                                            start=(s == 0 and t == 0),
                                            stop=False,
                                        )
                                    if s == N_SRC_WIN - 1:
                                        nc.tensor.matmul(
                                            bank[:, :],
                                            ident_sb[:, :],
                                            staged_prev[:, w, :],
                                            start=False,
                                            stop=True,
                                        )
                            if pending is not None:
                                drain_super(pending[0], pending[1], staged)
                                if pending[0] == SPLIT_SUP - 1:
                                    send_piece(hop, staged, 0, WSPLIT)
                            pending = (sup, banks)
                    drain_super(pending[0], pending[1], staged)
                    send_piece(hop, staged, WSPLIT, W_PER_CORE)
                    gate_accum(staged)
                    staged_prev = staged

            # ---- final: out = acc * sqrtdeg (in place), then store ----
            for w in range(W_PER_CORE):
                nc.vector.tensor_scalar_mul(
                    acc_sb[:, w, :], acc_sb[:, w, :], sqd_sb[:, w : w + 1]
                )
            nc.sync.dma_start(
                out=out_pm[:, :].rearrange("(p w) f -> p (w f)", p=128),
                in_=acc_sb[:, :, :],
            )

    nc.finalize()
    return nc


# ----------------------------------------------------------------------------
# Entry point
# ----------------------------------------------------------------------------
_CACHED = {}


def kernel(**inputs):
    feats = np.asarray(inputs["feats"], dtype=np.float32)
    s = np.asarray(inputs["s"], dtype=np.float32)
    src = np.asarray(inputs["src"])
    dst = np.asarray(inputs["dst"])

    in_maps, meta = _preprocess(feats, s, src, dst)

    from concourse.bass_utils import run_bass_kernel_spmd

    nc = _CACHED.get("nc")
    if nc is None:
        nc = _build()
        _CACHED["nc"] = nc

    res = run_bass_kernel_spmd(nc, in_maps, core_ids=list(range(N_CORES)))
    _CACHED["last_result"] = res
    out = np.zeros((N_NODES, D), dtype=np.float32)
    core_of, wloc_of, pos_of = meta["core_of"], meta["wloc_of"], meta["pos_of"]
    rows = pos_of.astype(np.int64) * W_PER_CORE + wloc_of.astype(np.int64)
    for c in range(N_CORES):
        m = core_of == c
        out[m] = res.results[c]["out_pm"][rows[m]]
    return out


if __name__ == "__main__":
    nc = _build()
    print("build ok")


# revision 27
# speedup vs baseline: 1.5428x; 1.0523x over previous
"""DAGNN (10-hop propagation + sigmoid gating) Bass kernel for 8 trn2 NeuronCores.

Strategy (1D node partition, SPMD-uniform schedule):
  - Host assigns nodes to (core, window, slot) with degree balancing so every
    core runs an identical instruction stream (one NEFF, 8 cores).
  - Node features live in a Shared DRAM replica as 256B rows: cols 0:64 hold
    bf16 values, cols 64:128 are never read (gather elem_size must be a
    multiple of 256B).
  - Per hop: dma_gather pulls per-edge rows (4 supers = 16 windows merged per
    gather instruction to amortize gpsimd descriptor-gen overhead); PE
    computes the segment-sum via one-hot indicator matmuls accumulating in
    fp32 PSUM. Indicators are host-precomputed fp8 tables streamed from DRAM
    (hop-invariant), so the Vector engine does no indicator work. The Scalar
    engine drains PSUM with deg^-1 scaling straight to bf16. A split
    AllGather (windows 0:48 early, 48:112 late) rebuilds the replica while
    the second half of the hop still computes.
  - Gating is fused into the hop loop: z_k = sum_f g_k*s, sig_k =
    sigmoid(z_k*sqrt(deg)), acc += sig_k * g_k — no final reload pass.
"""

import sys

sys.path.insert(0, "/opt/trn_rl_repo")

import numpy as np
import ml_dtypes

BF16 = ml_dtypes.bfloat16
FP8 = ml_dtypes.float8_e4m3fn


def _config(n_nodes, k_hops, n_cores, w_per_core, w_per_super, t_per_bucket):
    g = globals()
    g["N_NODES"] = n_nodes
    g["D"] = 64
    g["K_HOPS"] = k_hops
    g["N_CORES"] = n_cores
    g["WIN"] = 128
    g["W_PER_CORE"] = w_per_core
    assert w_per_core * n_cores * 128 >= n_nodes
    g["ROWS_PC"] = w_per_core * 128
    g["REP_ROWS"] = n_cores * g["ROWS_PC"]
    g["N_SRC_WIN"] = 4
    assert g["REP_ROWS"] % 4 == 0
    g["SRC_WIN"] = g["REP_ROWS"] // 4
    assert g["SRC_WIN"] <= 32768
    g["W_PER_SUPER"] = w_per_super
    g["SUPERS_PER_GRP"] = 2
    g["W_PER_GRP"] = w_per_super * g["SUPERS_PER_GRP"]
    assert w_per_core % g["W_PER_GRP"] == 0
    g["SUPERS"] = w_per_core // w_per_super
    g["GROUPS"] = w_per_core // g["W_PER_GRP"]
    g["T_PER_BUCKET"] = t_per_bucket
    g["SLOTS_PER_WS"] = t_per_bucket * 128
    # per (group, src-window) gather block
    g["GRP_SLOTS"] = g["W_PER_GRP"] * g["SLOTS_PER_WS"]
    g["GRP_TILES"] = g["GRP_SLOTS"] // 128
    g["SLOTS_TOTAL"] = w_per_core * 4 * g["SLOTS_PER_WS"]
    g["TILES_TOTAL"] = g["SLOTS_TOTAL"] // 128


_config(100000, 10, 8, 104, 4, 3)
# replica is split into N_SRC_WIN piece tensors of W_PIECE windows each; a
# piece is AllGathered as soon as its windows drain, and is exactly one
# gather source window.
W_PIECE = W_PER_CORE // N_SRC_WIN
ROWS_PIECE = W_PIECE * 128
assert N_CORES * ROWS_PIECE == SRC_WIN


# ----------------------------------------------------------------------------
# Host preprocessing
# ----------------------------------------------------------------------------
def _balance_assign(deg_s_fn, tot):
    """Assign nodes to global windows (N_CORES*W_PER_CORE, cap 128 each) so
    that every (window, src-window) edge count stays <= SLOTS_PER_WS."""
    import heapq

    n = tot.shape[0]
    n_windows = N_CORES * W_PER_CORE
    order = np.argsort(-tot, kind="stable")
    heap = [(0, w) for w in range(n_windows)]
    heapq.heapify(heap)
    win_of = np.empty(n, dtype=np.int32)
    win_fill = np.zeros(n_windows, dtype=np.int32)
    for v in order:
        while True:
            load, w = heapq.heappop(heap)
            if win_fill[w] < WIN:
                break
        win_of[v] = w
        win_fill[w] += 1
        if win_fill[w] < WIN:
            heapq.heappush(heap, (load + int(tot[v]), w))

    rng = np.random.default_rng(12345)
    cap = SLOTS_PER_WS
    for round_i in range(12):
        pos_of = np.zeros(n, dtype=np.int32)
        ordv = np.lexsort((np.arange(n), win_of))
        posctr = np.zeros(n_windows, dtype=np.int32)
        for v in ordv:
            pos_of[v] = posctr[win_of[v]]
            posctr[win_of[v]] += 1
        core_of = (win_of // W_PER_CORE).astype(np.int32)
        wloc_of = (win_of % W_PER_CORE).astype(np.int32)
        deg_s = deg_s_fn(core_of, wloc_of, pos_of)  # [n, 4]
        loads = np.zeros((n_windows, N_SRC_WIN), dtype=np.int64)
        np.add.at(loads, win_of, deg_s)
        over = np.flatnonzero((loads > cap).any(axis=1))
        if len(over) == 0:
            return core_of, wloc_of, pos_of
        for w in over:
            s_bad = int(np.argmax(loads[w]))
            excess = int(loads[w, s_bad] - cap)
            members = np.flatnonzero(win_of == w)
            mdeg = deg_s[members, s_bad]
            for v in members[np.argsort(-mdeg)]:
                if excess <= 0:
                    break
                cands = rng.integers(0, n_windows, 64)
                best, bestval = -1, None
                for cw in cands:
                    if cw == w or posctr[cw] >= WIN:
                        continue
                    val = int((loads[cw] + deg_s[v]).max())
                    if val <= cap - 8 and (bestval is None or val < bestval):
                        best, bestval = int(cw), val
                if best < 0:
                    continue
                loads[w] -= deg_s[v]
                loads[best] += deg_s[v]
                win_of[v] = best
                posctr[w] -= 1
                posctr[best] += 1
                excess -= int(deg_s[v, s_bad])
    raise RuntimeError("balance repair failed to converge")


def _preprocess(feats, s, src, dst):
    src = np.asarray(src, dtype=np.int64)
    dst = np.asarray(dst, dtype=np.int64)
    n = N_NODES
    deg = np.bincount(dst, minlength=n).astype(np.float64)
    n2 = (1.0 / deg).astype(np.float32)
    norm = (deg ** -0.5).astype(np.float32)
    sqrtdeg = np.sqrt(deg).astype(np.float32)

    # ---- peel one self-loop per node (handled via identity matmul) ----
    loop_mask = src == dst
    loop_idx = np.flatnonzero(loop_mask)
    uniq_nodes, first_pos = np.unique(dst[loop_idx], return_index=True)
    if len(uniq_nodes) != n:
        raise RuntimeError("not every node has a self-loop; identity fold invalid")
    drop = np.zeros(len(src), dtype=bool)
    drop[loop_idx[first_pos]] = True
    src = src[~drop]
    dst = dst[~drop]

    # ---- node assignment (core, window, pos) ----
    deg_r = np.bincount(dst, minlength=n).astype(np.int64)

    def split_row(core_of, wloc_of, pos_of):
        """(src window id 0..3, row within window): piece s holds windows
        [s*W_PIECE, (s+1)*W_PIECE), w-major rows, core-major globally."""
        c = core_of.astype(np.int64)
        w = wloc_of.astype(np.int64)
        p = pos_of.astype(np.int64)
        s = w // W_PIECE
        row = c * ROWS_PIECE + (w - s * W_PIECE) * 128 + p
        return s, row

    def deg_s_fn(core_of, wloc_of, pos_of):
        es, _ = split_row(core_of[src], wloc_of[src], pos_of[src])
        out = np.zeros((n, N_SRC_WIN), dtype=np.int64)
        np.add.at(out, (dst, es), 1)
        return out

    core_of, wloc_of, pos_of = _balance_assign(deg_s_fn, deg_r)

    # ---- per-core edge bucketing (group = 4 supers merged per gather) ----
    e_core = core_of[dst]
    e_w = wloc_of[dst]
    e_key = pos_of[dst]
    e_s, e_gidx = split_row(core_of[src], wloc_of[src], pos_of[src])

    gidx_all = np.zeros((N_CORES, SLOTS_TOTAL), dtype=np.int16)
    keys_all = np.full((N_CORES, SLOTS_TOTAL), -1, dtype=np.int32)

    for c in range(N_CORES):
        m = e_core == c
        cw = e_w[m]
        cs = e_s[m]
        ckey = e_key[m]
        cg = e_gidx[m]
        ws = cw * N_SRC_WIN + cs
        order = np.argsort(ws, kind="stable")
        cw, cs, ckey, cg, ws = cw[order], cs[order], ckey[order], cg[order], ws[order]
        counts = np.bincount(ws, minlength=W_PER_CORE * N_SRC_WIN)
        if counts.max() > SLOTS_PER_WS:
            raise RuntimeError(f"bucket overflow: {counts.max()} > {SLOTS_PER_WS}")
        # slot layout: (g, s) block of GRP_SLOTS; within it
        # sl (super in group) * 4*SLOTS_PER_WS + wi*SLOTS_PER_WS + runpos
        w_arr = np.arange(W_PER_CORE * N_SRC_WIN) // N_SRC_WIN
        s_arr = np.arange(W_PER_CORE * N_SRC_WIN) % N_SRC_WIN
        g_arr = w_arr // W_PER_GRP
        sl_arr = (w_arr % W_PER_GRP) // W_PER_SUPER
        wi_arr = w_arr % W_PER_SUPER
        starts = (
            (g_arr * N_SRC_WIN + s_arr) * GRP_SLOTS
            + sl_arr * (W_PER_SUPER * SLOTS_PER_WS)
            + wi_arr * SLOTS_PER_WS
        )
        runpos = np.arange(len(ws)) - np.repeat(
            np.concatenate([[0], np.cumsum(counts)[:-1]]), counts
        )
        slots = starts[ws] + runpos
        gidx_all[c, slots] = cg.astype(np.int16)
        keys_all[c, slots] = ckey.astype(np.int32)

    # ---- initial replica g0 = norm * feats in bf16, 256B rows, w-major ----
    g0 = (feats.astype(np.float32) * norm[:, None]).astype(BF16)
    g0_rep = np.zeros((REP_ROWS, 2 * D), dtype=BF16)
    own_row = (
        core_of.astype(np.int64) * ROWS_PC
        + wloc_of.astype(np.int64) * 128
        + pos_of.astype(np.int64)
    )
    g0_rep[own_row, 0:D] = g0

    def pw_table(vec):
        out = np.zeros((N_CORES, WIN, W_PER_CORE), dtype=np.float32)
        out[core_of, pos_of, wloc_of] = vec
        return out

    n2_pw = pw_table(n2)
    n2_pw[n2_pw == 0] = 1.0
    sqd_pw = pw_table(sqrtdeg)

    gidx_wrapped = np.ascontiguousarray(
        np.tile(
            gidx_all.reshape(N_CORES, SLOTS_TOTAL // 16, 16).transpose(0, 2, 1),
            (1, 8, 1),
        )
    )
    # indicator one-hot tables, fp8: ind[p, tile, f] = (key[slot=tile*128+p] == f)
    keys_pt = keys_all.reshape(N_CORES, TILES_TOTAL, 128).transpose(0, 2, 1)
    fvals = np.arange(128, dtype=np.int32)
    ind_all = (keys_pt[:, :, :, None] == fvals).astype(FP8)  # [C,128,TILES,128]
    ind_all = ind_all.reshape(N_CORES, 128, TILES_TOTAL * 128)

    ident8 = np.ascontiguousarray(np.eye(128, dtype=np.float32).astype(FP8))
    s_bcast = np.broadcast_to(
        np.asarray(s, dtype=np.float32).reshape(1, D), (128, D)
    ).astype(BF16)
    s_bcast = np.ascontiguousarray(s_bcast)

    in_maps = []
    for c in range(N_CORES):
        in_maps.append(
            {
                "g0_own": np.ascontiguousarray(
                    g0_rep[c * ROWS_PC : (c + 1) * ROWS_PC]
                ),
                "gidx": gidx_wrapped[c],
                "indt": np.ascontiguousarray(ind_all[c]),
                "n2_pw": np.ascontiguousarray(n2_pw[c]),
                "sqd_pw": np.ascontiguousarray(sqd_pw[c]),
                "s_bcast": s_bcast,
                "ident8": ident8,
            }
        )
    meta = {
        "core_of": core_of,
        "wloc_of": wloc_of,
        "pos_of": pos_of,
    }
    return in_maps, meta


# ----------------------------------------------------------------------------
# Bass kernel builder (identical program for all cores)
# ----------------------------------------------------------------------------
def _build():
    import concourse.bacc as bacc
    import concourse.mybir as mybir
    from concourse.tile import TileContext

    fp32 = mybir.dt.float32
    bf16 = mybir.dt.bfloat16
    fp8 = mybir.dt.float8e4
    i16 = mybir.dt.int16

    nc = bacc.Bacc(None, target_bir_lowering=False, num_devices=N_CORES, num_swdge_queues=4)

    # I/O
    g0_own = nc.dram_tensor("g0_own", [ROWS_PC, 2 * D], bf16, kind="ExternalInput")
    gidx_in = nc.dram_tensor("gidx", [128, SLOTS_TOTAL // 16], i16, kind="ExternalInput")
    indt_in = nc.dram_tensor("indt", [128, TILES_TOTAL * 128], fp8, kind="ExternalInput")
    n2_in = nc.dram_tensor("n2_pw", [128, W_PER_CORE], fp32, kind="ExternalInput")
    sqd_in = nc.dram_tensor("sqd_pw", [128, W_PER_CORE], fp32, kind="ExternalInput")
    s_in = nc.dram_tensor("s_bcast", [128, D], bf16, kind="ExternalInput")
    ident_in = nc.dram_tensor("ident8", [128, 128], fp8, kind="ExternalInput")
    out_pm = nc.dram_tensor("out_pm", [ROWS_PC, D], fp32, kind="ExternalOutput")

    cc_in_p = [
        [
            nc.dram_tensor(f"cc_in_{s}_{k}", [ROWS_PIECE, 2 * D], bf16)
            for k in range(K_HOPS)
        ]
        for s in range(N_SRC_WIN)
    ]
    cc_out_p = [
        [
            nc.dram_tensor(
                f"cc_out_{s}_{k}", [SRC_WIN, 2 * D], bf16, addr_space="Shared"
            )
            for k in range(K_HOPS)
        ]
        for s in range(N_SRC_WIN)
    ]
    groups = [list(range(N_CORES))]

    with TileContext(nc) as tc:
        with tc.tile_pool(name="const", bufs=1) as const_pool:
            # ---- load static tables ----
            gidx_sb = const_pool.tile([128, SLOTS_TOTAL // 16], i16, tag="gidx")
            nc.sync.dma_start(out=gidx_sb[:, :], in_=gidx_in[:, :])
            n2_sb = const_pool.tile([128, W_PER_CORE], fp32, tag="n2")
            nc.sync.dma_start(out=n2_sb[:, :], in_=n2_in[:, :])
            sqd_sb = const_pool.tile([128, W_PER_CORE], fp32, tag="sqd")
            nc.sync.dma_start(out=sqd_sb[:, :], in_=sqd_in[:, :])
            s_sb = const_pool.tile([128, D], bf16, tag="svec")
            nc.sync.dma_start(out=s_sb[:, :], in_=s_in[:, :])
            ident_sb = const_pool.tile([128, 128], fp8, tag="ident")
            nc.sync.dma_start(out=ident_sb[:, :], in_=ident_in[:, :])

            # gating state
            zt_sb = const_pool.tile([128, W_PER_CORE], fp32, tag="zt")
            sig_sb = const_pool.tile([128, W_PER_CORE], fp32, tag="sig")
            acc_sb = const_pool.tile([128, W_PER_CORE, D], fp32, tag="acc")
            zuf_sb = const_pool.tile([128, W_PER_CORE, D], bf16, tag="zuf")
            nc.vector.memset(acc_sb[:, :, :], 0.0)

            def gate_accum(staged):
                nc.vector.tensor_tensor(
                    zuf_sb[:, :, :],
                    staged[:, :, :],
                    s_sb[:, :]
                    .rearrange("p (one f) -> p one f", one=1)
                    .broadcast_to((128, W_PER_CORE, D)),
                    mybir.AluOpType.mult,
                )
                nc.vector.tensor_reduce(
                    zt_sb[:, :],
                    zuf_sb[:, :, :],
                    mybir.AxisListType.X,
                    mybir.AluOpType.add,
                )
                nc.vector.tensor_tensor(
                    zt_sb[:, :], zt_sb[:, :], sqd_sb[:, :], mybir.AluOpType.mult
                )
                nc.scalar.activation(
                    sig_sb[:, :],
                    zt_sb[:, :],
                    mybir.ActivationFunctionType.Sigmoid,
                )
                for w in range(W_PER_CORE):
                    nc.vector.scalar_tensor_tensor(
                        acc_sb[:, w, :],
                        staged[:, w, :],
                        sig_sb[:, w : w + 1],
                        acc_sb[:, w, :],
                        mybir.AluOpType.mult,
                        mybir.AluOpType.add,
                    )

            with (
                tc.tile_pool(name="chunks", bufs=12) as chunk_pool,
                tc.tile_pool(name="inds", bufs=2) as ind_pool,
                tc.tile_pool(name="stage", bufs=2) as stage_pool,
                tc.tile_pool(name="psum", bufs=8, space="PSUM") as psum_pool,
            ):

                # ---- bootstrap ----
                for s in range(N_SRC_WIN):
                    nc.sync.dma_start(
                        out=cc_in_p[s][0][:, :],
                        in_=g0_own[s * ROWS_PIECE : (s + 1) * ROWS_PIECE, :],
                    )
                    nc.gpsimd.collective_compute(
                        "AllGather",
                        mybir.AluOpType.bypass,
                        replica_groups=groups,
                        ins=[cc_in_p[s][0][:, :]],
                        outs=[cc_out_p[s][0][:, :]],
                    )
                staged_prev = stage_pool.tile([128, W_PER_CORE, D], bf16, tag="staged")
                nc.sync.dma_start(
                    out=staged_prev[:, :, :],
                    in_=g0_own[:, 0:D].rearrange("(w p) f -> p w f", p=128),
                )
                gate_accum(staged_prev)

                def drain_super(sup, banks, staged):
                    for wi in range(W_PER_SUPER):
                        w = sup * W_PER_SUPER + wi
                        nc.scalar.activation(
                            staged[:, w, :],
                            banks[wi][:, :],
                            mybir.ActivationFunctionType.Copy,
                            scale=n2_sb[:, w : w + 1],
                        )

                def send_piece(hop, staged, s):
                    """DMA staged windows of piece s into cc_in_p[s], then
                    AllGather the whole (contiguous) piece tensor."""
                    if hop >= K_HOPS - 1:
                        return
                    w0 = s * W_PIECE
                    nc.sync.dma_start(
                        out=cc_in_p[s][hop + 1][:, 0:D].rearrange(
                            "(w p) f -> p w f", p=128
                        ),
                        in_=staged[:, w0 : w0 + W_PIECE, :],
                    )
                    nc.gpsimd.collective_compute(
                        "AllGather",
                        mybir.AluOpType.bypass,
                        replica_groups=groups,
                        ins=[cc_in_p[s][hop + 1][:, :]],
                        outs=[cc_out_p[s][hop + 1][:, :]],
                    )

                # fire piece s once its last window has drained:
                # piece s covers windows [26s, 26s+26) -> last super ceil
                PIECE_AT = {6: 0, 12: 1, 19: 2, 25: 3}

                for hop in range(K_HOPS):
                    staged = stage_pool.tile([128, W_PER_CORE, D], bf16, tag="staged")
                    pending = None
                    for grp in range(GROUPS):
                        # all 4*GRP_TILES indicator tiles of this group in one DMA
                        itile0 = grp * N_SRC_WIN * GRP_TILES
                        indg = ind_pool.tile(
                            [128, N_SRC_WIN * GRP_TILES, 128], fp8, tag="ind"
                        )
                        nc.sync.dma_start(
                            out=indg[:, :, :],
                            in_=indt_in[
                                :,
                                itile0 * 128 : (itile0 + N_SRC_WIN * GRP_TILES) * 128,
                            ],
                        )
                        chunks = []
                        for s in range(N_SRC_WIN):
                            ch = chunk_pool.tile(
                                [128, GRP_TILES, 2 * D], bf16, tag="chunk"
                            )
                            col0 = (grp * N_SRC_WIN + s) * (GRP_SLOTS // 16)
                            src_rep = cc_out_p[s][hop]
                            nc.gpsimd.dma_gather(
                                ch[:, :, :],
                                src_rep[0:SRC_WIN, :],
                                gidx_sb[:, col0 : col0 + GRP_SLOTS // 16],
                                GRP_SLOTS,
                                GRP_SLOTS,
                                2 * D,
                                single_packet=False,
                                queue_num=s,
                            )
                            chunks.append(ch)
                        wt = W_PER_SUPER * T_PER_BUCKET
                        for sl in range(SUPERS_PER_GRP):
                            sup = grp * SUPERS_PER_GRP + sl
                            banks = [
                                psum_pool.tile([128, D], fp32, tag="bank", name="bank")
                                for _ in range(W_PER_SUPER)
                            ]
                            for s in range(N_SRC_WIN):
                                for wi in range(W_PER_SUPER):
                                    w = sup * W_PER_SUPER + wi
                                    bank = banks[wi]
                                    for t in range(T_PER_BUCKET):
                                        j = sl * wt + wi * T_PER_BUCKET + t
                                        nc.tensor.matmul(
                                            bank[:, :],
                                            indg[:, s * GRP_TILES + j, :],
                                            chunks[s][:, j, 0:D],
                                            start=(s == 0 and t == 0),
                                            stop=False,
                                        )
                                    if s == N_SRC_WIN - 1:
                                        nc.tensor.matmul(
                                            bank[:, :],
                                            ident_sb[:, :],
                                            staged_prev[:, w, :],
                                            start=False,
                                            stop=True,
                                        )
                            if pending is not None:
                                drain_super(pending[0], pending[1], staged)
                                if pending[0] in PIECE_AT:
                                    send_piece(hop, staged, PIECE_AT[pending[0]])
                            pending = (sup, banks)
                    drain_super(pending[0], pending[1], staged)
                    send_piece(hop, staged, PIECE_AT[pending[0]])
                    gate_accum(staged)
                    staged_prev = staged

            # ---- final: out = acc * sqrtdeg (in place), then store ----
            for w in range(W_PER_CORE):
                nc.vector.tensor_scalar_mul(
                    acc_sb[:, w, :], acc_sb[:, w, :], sqd_sb[:, w : w + 1]
                )
            nc.sync.dma_start(
                out=out_pm[:, :].rearrange("(p w) f -> p (w f)", p=128),
                in_=acc_sb[:, :, :],
            )

    nc.finalize()
    return nc


# ----------------------------------------------------------------------------
# Entry point
# ----------------------------------------------------------------------------
_CACHED = {}


def kernel(**inputs):
    feats = np.asarray(inputs["feats"], dtype=np.float32)
    s = np.asarray(inputs["s"], dtype=np.float32)
    src = np.asarray(inputs["src"])
    dst = np.asarray(inputs["dst"])

    in_maps, meta = _preprocess(feats, s, src, dst)

    from concourse.bass_utils import run_bass_kernel_spmd

    nc = _CACHED.get("nc")
    if nc is None:
        nc = _build()
        _CACHED["nc"] = nc

    res = run_bass_kernel_spmd(nc, in_maps, core_ids=list(range(N_CORES)))
    _CACHED["last_result"] = res
    out = np.zeros((N_NODES, D), dtype=np.float32)
    core_of, wloc_of, pos_of = meta["core_of"], meta["wloc_of"], meta["pos_of"]
    rows = pos_of.astype(np.int64) * W_PER_CORE + wloc_of.astype(np.int64)
    for c in range(N_CORES):
        m = core_of == c
        out[m] = res.results[c]["out_pm"][rows[m]]
    return out


if __name__ == "__main__":
    nc = _build()
    print("build ok")


# revision 33
# speedup vs baseline: 1.6828x; 1.0908x over previous
"""DAGNN (10-hop propagation + sigmoid gating) Bass kernel for 8 trn2 NeuronCores.

Strategy (1D node partition, SPMD-uniform schedule):
  - Host assigns nodes to (core, window, slot) with degree balancing so every
    core runs an identical instruction stream (one NEFF, 8 cores).
  - Node features live in a Shared DRAM replica as 256B rows: cols 0:64 hold
    bf16 values, cols 64:128 are never read (gather elem_size must be a
    multiple of 256B).
  - Per hop: dma_gather pulls per-edge rows (4 supers = 16 windows merged per
    gather instruction to amortize gpsimd descriptor-gen overhead); PE
    computes the segment-sum via one-hot indicator matmuls accumulating in
    fp32 PSUM. Indicators are host-precomputed fp8 tables streamed from DRAM
    (hop-invariant), so the Vector engine does no indicator work. The Scalar
    engine drains PSUM with deg^-1 scaling straight to bf16. A split
    AllGather (windows 0:48 early, 48:112 late) rebuilds the replica while
    the second half of the hop still computes.
  - Gating is fused into the hop loop: z_k = sum_f g_k*s, sig_k =
    sigmoid(z_k*sqrt(deg)), acc += sig_k * g_k — no final reload pass.
"""

import sys

sys.path.insert(0, "/opt/trn_rl_repo")

import numpy as np
import ml_dtypes

BF16 = ml_dtypes.bfloat16
FP8 = ml_dtypes.float8_e4m3fn


def _config(n_nodes, k_hops, n_cores, w_per_core, w_per_super, t_per_bucket):
    g = globals()
    g["N_NODES"] = n_nodes
    g["D"] = 64
    g["K_HOPS"] = k_hops
    g["N_CORES"] = n_cores
    g["WIN"] = 128
    g["W_PER_CORE"] = w_per_core
    assert w_per_core * n_cores * 128 >= n_nodes
    g["ROWS_PC"] = w_per_core * 128
    g["REP_ROWS"] = n_cores * g["ROWS_PC"]
    g["N_SRC_WIN"] = 4
    assert g["REP_ROWS"] % 4 == 0
    g["SRC_WIN"] = g["REP_ROWS"] // 4
    assert g["SRC_WIN"] <= 32768
    g["W_PER_SUPER"] = w_per_super
    g["SUPERS_PER_GRP"] = 2
    g["W_PER_GRP"] = w_per_super * g["SUPERS_PER_GRP"]
    assert w_per_core % g["W_PER_GRP"] == 0
    g["SUPERS"] = w_per_core // w_per_super
    g["GROUPS"] = w_per_core // g["W_PER_GRP"]
    g["T_PER_BUCKET"] = t_per_bucket
    g["SLOTS_PER_WS"] = t_per_bucket * 128
    # per (group, src-window) gather block
    g["GRP_SLOTS"] = g["W_PER_GRP"] * g["SLOTS_PER_WS"]
    g["GRP_TILES"] = g["GRP_SLOTS"] // 128
    g["SLOTS_TOTAL"] = w_per_core * 4 * g["SLOTS_PER_WS"]
    g["TILES_TOTAL"] = g["SLOTS_TOTAL"] // 128


_config(100000, 10, 8, 104, 4, 3)
# replica is split into N_SRC_WIN piece tensors of W_PIECE windows each; a
# piece is AllGathered as soon as its windows drain, and is exactly one
# gather source window.
W_PIECE = W_PER_CORE // N_SRC_WIN
ROWS_PIECE = W_PIECE * 128
assert N_CORES * ROWS_PIECE == SRC_WIN


# ----------------------------------------------------------------------------
# Host preprocessing
# ----------------------------------------------------------------------------
def _balance_assign(deg_s_fn, tot):
    """Assign nodes to global windows (N_CORES*W_PER_CORE, cap 128 each) so
    that every (window, src-window) edge count stays <= SLOTS_PER_WS."""
    import heapq

    n = tot.shape[0]
    n_windows = N_CORES * W_PER_CORE
    order = np.argsort(-tot, kind="stable")
    heap = [(0, w) for w in range(n_windows)]
    heapq.heapify(heap)
    win_of = np.empty(n, dtype=np.int32)
    win_fill = np.zeros(n_windows, dtype=np.int32)
    for v in order:
        while True:
            load, w = heapq.heappop(heap)
            if win_fill[w] < WIN:
                break
        win_of[v] = w
        win_fill[w] += 1
        if win_fill[w] < WIN:
            heapq.heappush(heap, (load + int(tot[v]), w))

    rng = np.random.default_rng(12345)
    cap = SLOTS_PER_WS
    for round_i in range(12):
        pos_of = np.zeros(n, dtype=np.int32)
        ordv = np.lexsort((np.arange(n), win_of))
        posctr = np.zeros(n_windows, dtype=np.int32)
        for v in ordv:
            pos_of[v] = posctr[win_of[v]]
            posctr[win_of[v]] += 1
        core_of = (win_of // W_PER_CORE).astype(np.int32)
        wloc_of = (win_of % W_PER_CORE).astype(np.int32)
        deg_s = deg_s_fn(core_of, wloc_of, pos_of)  # [n, 4]
        loads = np.zeros((n_windows, N_SRC_WIN), dtype=np.int64)
        np.add.at(loads, win_of, deg_s)
        over = np.flatnonzero((loads > cap).any(axis=1))
        if len(over) == 0:
            return core_of, wloc_of, pos_of
        for w in over:
            s_bad = int(np.argmax(loads[w]))
            excess = int(loads[w, s_bad] - cap)
            members = np.flatnonzero(win_of == w)
            mdeg = deg_s[members, s_bad]
            for v in members[np.argsort(-mdeg)]:
                if excess <= 0:
                    break
                cands = rng.integers(0, n_windows, 64)
                best, bestval = -1, None
                for cw in cands:
                    if cw == w or posctr[cw] >= WIN:
                        continue
                    val = int((loads[cw] + deg_s[v]).max())
                    if val <= cap - 8 and (bestval is None or val < bestval):
                        best, bestval = int(cw), val
                if best < 0:
                    continue
                loads[w] -= deg_s[v]
                loads[best] += deg_s[v]
                win_of[v] = best
                posctr[w] -= 1
                posctr[best] += 1
                excess -= int(deg_s[v, s_bad])
    raise RuntimeError("balance repair failed to converge")


def _preprocess(feats, s, src, dst):
    src = np.asarray(src, dtype=np.int64)
    dst = np.asarray(dst, dtype=np.int64)
    n = N_NODES
    deg = np.bincount(dst, minlength=n).astype(np.float64)
    n2 = (1.0 / deg).astype(np.float32)
    norm = (deg ** -0.5).astype(np.float32)
    sqrtdeg = np.sqrt(deg).astype(np.float32)

    # ---- peel one self-loop per node (handled via identity matmul) ----
    loop_mask = src == dst
    loop_idx = np.flatnonzero(loop_mask)
    uniq_nodes, first_pos = np.unique(dst[loop_idx], return_index=True)
    if len(uniq_nodes) != n:
        raise RuntimeError("not every node has a self-loop; identity fold invalid")
    drop = np.zeros(len(src), dtype=bool)
    drop[loop_idx[first_pos]] = True
    src = src[~drop]
    dst = dst[~drop]

    # ---- node assignment (core, window, pos) ----
    deg_r = np.bincount(dst, minlength=n).astype(np.int64)

    def split_row(core_of, wloc_of, pos_of):
        """(src window id 0..3, row within window): piece s holds windows
        [s*W_PIECE, (s+1)*W_PIECE), w-major rows, core-major globally."""
        c = core_of.astype(np.int64)
        w = wloc_of.astype(np.int64)
        p = pos_of.astype(np.int64)
        s = w // W_PIECE
        row = c * ROWS_PIECE + (w - s * W_PIECE) * 128 + p
        return s, row

    def deg_s_fn(core_of, wloc_of, pos_of):
        es, _ = split_row(core_of[src], wloc_of[src], pos_of[src])
        out = np.zeros((n, N_SRC_WIN), dtype=np.int64)
        np.add.at(out, (dst, es), 1)
        return out

    core_of, wloc_of, pos_of = _balance_assign(deg_s_fn, deg_r)

    # ---- per-core edge bucketing (group = 4 supers merged per gather) ----
    e_core = core_of[dst]
    e_w = wloc_of[dst]
    e_key = pos_of[dst]
    e_s, e_gidx = split_row(core_of[src], wloc_of[src], pos_of[src])

    gidx_all = np.zeros((N_CORES, SLOTS_TOTAL), dtype=np.int16)
    keys_all = np.full((N_CORES, SLOTS_TOTAL), -1, dtype=np.int32)

    for c in range(N_CORES):
        m = e_core == c
        cw = e_w[m]
        cs = e_s[m]
        ckey = e_key[m]
        cg = e_gidx[m]
        ws = cw * N_SRC_WIN + cs
        order = np.argsort(ws, kind="stable")
        cw, cs, ckey, cg, ws = cw[order], cs[order], ckey[order], cg[order], ws[order]
        counts = np.bincount(ws, minlength=W_PER_CORE * N_SRC_WIN)
        if counts.max() > SLOTS_PER_WS:
            raise RuntimeError(f"bucket overflow: {counts.max()} > {SLOTS_PER_WS}")
        # slot layout: (g, s) block of GRP_SLOTS; within it
        # sl (super in group) * 4*SLOTS_PER_WS + wi*SLOTS_PER_WS + runpos
        w_arr = np.arange(W_PER_CORE * N_SRC_WIN) // N_SRC_WIN
        s_arr = np.arange(W_PER_CORE * N_SRC_WIN) % N_SRC_WIN
        g_arr = w_arr // W_PER_GRP
        sl_arr = (w_arr % W_PER_GRP) // W_PER_SUPER
        wi_arr = w_arr % W_PER_SUPER
        starts = (
            (g_arr * N_SRC_WIN + s_arr) * GRP_SLOTS
            + sl_arr * (W_PER_SUPER * SLOTS_PER_WS)
            + wi_arr * SLOTS_PER_WS
        )
        runpos = np.arange(len(ws)) - np.repeat(
            np.concatenate([[0], np.cumsum(counts)[:-1]]), counts
        )
        slots = starts[ws] + runpos
        gidx_all[c, slots] = cg.astype(np.int16)
        keys_all[c, slots] = ckey.astype(np.int32)

    # ---- initial replica g0 = norm * feats in bf16, 256B rows, w-major ----
    g0 = (feats.astype(np.float32) * norm[:, None]).astype(BF16)
    g0_rep = np.zeros((REP_ROWS, 2 * D), dtype=BF16)
    own_row = (
        core_of.astype(np.int64) * ROWS_PC
        + wloc_of.astype(np.int64) * 128
        + pos_of.astype(np.int64)
    )
    g0_rep[own_row, 0:D] = g0

    def pw_table(vec):
        out = np.zeros((N_CORES, WIN, W_PER_CORE), dtype=np.float32)
        out[core_of, pos_of, wloc_of] = vec
        return out

    n2_pw = pw_table(n2)
    n2_pw[n2_pw == 0] = 1.0
    sqd_pw = pw_table(sqrtdeg)

    gidx_wrapped = np.ascontiguousarray(
        np.tile(
            gidx_all.reshape(N_CORES, SLOTS_TOTAL // 16, 16).transpose(0, 2, 1),
            (1, 8, 1),
        )
    )
    # keys layout [128, TILES_TOTAL]: slot j -> (j%128, j//128)
    keys_tiles = np.ascontiguousarray(
        keys_all.reshape(N_CORES, TILES_TOTAL, 128).transpose(0, 2, 1).astype(BF16)
    )
    iota = np.ascontiguousarray(
        np.broadcast_to(np.arange(128, dtype=np.float32), (128, 128)).astype(BF16)
    )

    ident8 = np.ascontiguousarray(np.eye(128, dtype=np.float32).astype(FP8))
    s_bcast = np.broadcast_to(
        np.asarray(s, dtype=np.float32).reshape(1, D), (128, D)
    ).astype(BF16)
    s_bcast = np.ascontiguousarray(s_bcast)

    in_maps = []
    for c in range(N_CORES):
        in_maps.append(
            {
                "g0_own": np.ascontiguousarray(
                    g0_rep[c * ROWS_PC : (c + 1) * ROWS_PC]
                ),
                "gidx": gidx_wrapped[c],
                "keys": keys_tiles[c],
                "iota": iota,
                "n2_pw": np.ascontiguousarray(n2_pw[c]),
                "sqd_pw": np.ascontiguousarray(sqd_pw[c]),
                "s_bcast": s_bcast,
                "ident8": ident8,
            }
        )
    meta = {
        "core_of": core_of,
        "wloc_of": wloc_of,
        "pos_of": pos_of,
    }
    return in_maps, meta


# ----------------------------------------------------------------------------
# Bass kernel builder (identical program for all cores)
# ----------------------------------------------------------------------------
def _build():
    import concourse.bacc as bacc
    import concourse.mybir as mybir
    from concourse.tile import TileContext

    fp32 = mybir.dt.float32
    bf16 = mybir.dt.bfloat16
    fp8 = mybir.dt.float8e4
    i16 = mybir.dt.int16

    nc = bacc.Bacc(None, target_bir_lowering=False, num_devices=N_CORES, num_swdge_queues=4)

    # I/O
    g0_own = nc.dram_tensor("g0_own", [ROWS_PC, 2 * D], bf16, kind="ExternalInput")
    gidx_in = nc.dram_tensor("gidx", [128, SLOTS_TOTAL // 16], i16, kind="ExternalInput")
    keys_in = nc.dram_tensor("keys", [128, TILES_TOTAL], bf16, kind="ExternalInput")
    iota_in = nc.dram_tensor("iota", [128, 128], bf16, kind="ExternalInput")
    n2_in = nc.dram_tensor("n2_pw", [128, W_PER_CORE], fp32, kind="ExternalInput")
    sqd_in = nc.dram_tensor("sqd_pw", [128, W_PER_CORE], fp32, kind="ExternalInput")
    s_in = nc.dram_tensor("s_bcast", [128, D], bf16, kind="ExternalInput")
    ident_in = nc.dram_tensor("ident8", [128, 128], fp8, kind="ExternalInput")
    out_pm = nc.dram_tensor("out_pm", [ROWS_PC, D], fp32, kind="ExternalOutput")

    cc_in_p = [
        [
            nc.dram_tensor(f"cc_in_{s}_{k}", [ROWS_PIECE, 2 * D], bf16)
            for k in range(K_HOPS)
        ]
        for s in range(N_SRC_WIN)
    ]
    cc_out_p = [
        [
            nc.dram_tensor(
                f"cc_out_{s}_{k}", [SRC_WIN, 2 * D], bf16, addr_space="Shared"
            )
            for k in range(K_HOPS)
        ]
        for s in range(N_SRC_WIN)
    ]
    groups = [list(range(N_CORES))]

    with TileContext(nc) as tc:
        with tc.tile_pool(name="const", bufs=1) as const_pool:
            # ---- load static tables ----
            gidx_sb = const_pool.tile([128, SLOTS_TOTAL // 16], i16, tag="gidx")
            nc.sync.dma_start(out=gidx_sb[:, :], in_=gidx_in[:, :])
            keys_sb = const_pool.tile([128, TILES_TOTAL], bf16, tag="keys")
            nc.sync.dma_start(out=keys_sb[:, :], in_=keys_in[:, :])
            iota_sb = const_pool.tile([128, 128], bf16, tag="iota")
            nc.sync.dma_start(out=iota_sb[:, :], in_=iota_in[:, :])
            n2_sb = const_pool.tile([128, W_PER_CORE], fp32, tag="n2")
            nc.sync.dma_start(out=n2_sb[:, :], in_=n2_in[:, :])
            sqd_sb = const_pool.tile([128, W_PER_CORE], fp32, tag="sqd")
            nc.sync.dma_start(out=sqd_sb[:, :], in_=sqd_in[:, :])
            s_sb = const_pool.tile([128, D], bf16, tag="svec")
            nc.sync.dma_start(out=s_sb[:, :], in_=s_in[:, :])
            ident_sb = const_pool.tile([128, 128], fp8, tag="ident")
            nc.sync.dma_start(out=ident_sb[:, :], in_=ident_in[:, :])

            # gating state
            zt_sb = const_pool.tile([128, W_PER_CORE], fp32, tag="zt")
            sig_sb = const_pool.tile([128, W_PER_CORE], fp32, tag="sig")
            acc_sb = const_pool.tile([128, W_PER_CORE, D], fp32, tag="acc")
            zuf_sb = const_pool.tile([128, W_PER_CORE, D], bf16, tag="zuf")
            nc.vector.memset(acc_sb[:, :, :], 0.0)

            def gate_accum(staged):
                nc.vector.tensor_tensor(
                    zuf_sb[:, :, :],
                    staged[:, :, :],
                    s_sb[:, :]
                    .rearrange("p (one f) -> p one f", one=1)
                    .broadcast_to((128, W_PER_CORE, D)),
                    mybir.AluOpType.mult,
                )
                nc.vector.tensor_reduce(
                    zt_sb[:, :],
                    zuf_sb[:, :, :],
                    mybir.AxisListType.X,
                    mybir.AluOpType.add,
                )
                nc.vector.tensor_tensor(
                    zt_sb[:, :], zt_sb[:, :], sqd_sb[:, :], mybir.AluOpType.mult
                )
                nc.scalar.activation(
                    sig_sb[:, :],
                    zt_sb[:, :],
                    mybir.ActivationFunctionType.Sigmoid,
                )
                for w in range(W_PER_CORE):
                    nc.vector.scalar_tensor_tensor(
                        acc_sb[:, w, :],
                        staged[:, w, :],
                        sig_sb[:, w : w + 1],
                        acc_sb[:, w, :],
                        mybir.AluOpType.mult,
                        mybir.AluOpType.add,
                    )

            with (
                tc.tile_pool(name="chunks", bufs=12) as chunk_pool,
                tc.tile_pool(name="inds", bufs=9) as ind_pool,
                tc.tile_pool(name="stage", bufs=2) as stage_pool,
                tc.tile_pool(name="psum", bufs=8, space="PSUM") as psum_pool,
            ):

                # ---- bootstrap ----
                for s in range(N_SRC_WIN):
                    nc.sync.dma_start(
                        out=cc_in_p[s][0][:, :],
                        in_=g0_own[s * ROWS_PIECE : (s + 1) * ROWS_PIECE, :],
                    )
                    nc.gpsimd.collective_compute(
                        "AllGather",
                        mybir.AluOpType.bypass,
                        replica_groups=groups,
                        ins=[cc_in_p[s][0][:, :]],
                        outs=[cc_out_p[s][0][:, :]],
                    )
                staged_prev = stage_pool.tile([128, W_PER_CORE, D], bf16, tag="staged")
                nc.sync.dma_start(
                    out=staged_prev[:, :, :],
                    in_=g0_own[:, 0:D].rearrange("(w p) f -> p w f", p=128),
                )
                gate_accum(staged_prev)

                def drain_super(sup, banks, staged):
                    for wi in range(W_PER_SUPER):
                        w = sup * W_PER_SUPER + wi
                        nc.scalar.activation(
                            staged[:, w, :],
                            banks[wi][:, :],
                            mybir.ActivationFunctionType.Copy,
                            scale=n2_sb[:, w : w + 1],
                        )

                def send_piece(hop, staged, s):
                    """DMA staged windows of piece s into cc_in_p[s], then
                    AllGather the whole (contiguous) piece tensor."""
                    if hop >= K_HOPS - 1:
                        return
                    w0 = s * W_PIECE
                    nc.sync.dma_start(
                        out=cc_in_p[s][hop + 1][:, 0:D].rearrange(
                            "(w p) f -> p w f", p=128
                        ),
                        in_=staged[:, w0 : w0 + W_PIECE, :],
                    )
                    nc.gpsimd.collective_compute(
                        "AllGather",
                        mybir.AluOpType.bypass,
                        replica_groups=groups,
                        ins=[cc_in_p[s][hop + 1][:, :]],
                        outs=[cc_out_p[s][hop + 1][:, :]],
                    )

                # fire piece s once its last window has drained:
                # piece s covers windows [26s, 26s+26) -> last super ceil
                PIECE_AT = {6: 0, 12: 1, 19: 2, 25: 3}

                for hop in range(K_HOPS):
                    staged = stage_pool.tile([128, W_PER_CORE, D], bf16, tag="staged")
                    pending = None
                    for grp in range(GROUPS):
                        chunks = []
                        for s in range(N_SRC_WIN):
                            ch = chunk_pool.tile(
                                [128, GRP_TILES, 2 * D], bf16, tag="chunk"
                            )
                            col0 = (grp * N_SRC_WIN + s) * (GRP_SLOTS // 16)
                            src_rep = cc_out_p[s][hop]
                            nc.gpsimd.dma_gather(
                                ch[:, :, :],
                                src_rep[0:SRC_WIN, :],
                                gidx_sb[:, col0 : col0 + GRP_SLOTS // 16],
                                GRP_SLOTS,
                                GRP_SLOTS,
                                2 * D,
                                single_packet=False,
                                queue_num=s,
                            )
                            chunks.append(ch)
                        wt = W_PER_SUPER * T_PER_BUCKET
                        for sl in range(SUPERS_PER_GRP):
                            sup = grp * SUPERS_PER_GRP + sl
                            banks = [
                                psum_pool.tile([128, D], fp32, tag="bank", name="bank")
                                for _ in range(W_PER_SUPER)
                            ]
                            for s in range(N_SRC_WIN):
                                tile0 = (grp * N_SRC_WIN + s) * GRP_TILES + sl * wt
                                indb = ind_pool.tile([128, wt, 128], bf16, tag="ind")
                                nc.vector.tensor_tensor(
                                    indb[:, :, :],
                                    iota_sb[:, :]
                                    .rearrange("p (one f) -> p one f", one=1)
                                    .broadcast_to((128, wt, 128)),
                                    keys_sb[:, tile0 : tile0 + wt].broadcast_to(
                                        (128, wt, 128)
                                    ),
                                    mybir.AluOpType.is_equal,
                                )
                                for wi in range(W_PER_SUPER):
                                    w = sup * W_PER_SUPER + wi
                                    bank = banks[wi]
                                    for t in range(T_PER_BUCKET):
                                        j = sl * wt + wi * T_PER_BUCKET + t
                                        nc.tensor.matmul(
                                            bank[:, :],
                                            indb[:, wi * T_PER_BUCKET + t, :],
                                            chunks[s][:, j, 0:D],
                                            start=(s == 0 and t == 0),
                                            stop=False,
                                        )
                                    if s == N_SRC_WIN - 1:
                                        nc.tensor.matmul(
                                            bank[:, :],
                                            ident_sb[:, :],
                                            staged_prev[:, w, :],
                                            start=False,
                                            stop=True,
                                        )
                            if pending is not None:
                                drain_super(pending[0], pending[1], staged)
                                if pending[0] in PIECE_AT:
                                    send_piece(hop, staged, PIECE_AT[pending[0]])
                            pending = (sup, banks)
                    drain_super(pending[0], pending[1], staged)
                    send_piece(hop, staged, PIECE_AT[pending[0]])
                    gate_accum(staged)
                    staged_prev = staged

            # ---- final: out = acc * sqrtdeg (in place), then store ----
            for w in range(W_PER_CORE):
                nc.vector.tensor_scalar_mul(
                    acc_sb[:, w, :], acc_sb[:, w, :], sqd_sb[:, w : w + 1]
                )
            nc.sync.dma_start(
                out=out_pm[:, :].rearrange("(p w) f -> p (w f)", p=128),
                in_=acc_sb[:, :, :],
            )

    nc.finalize()
    return nc


# ----------------------------------------------------------------------------
# Entry point
# ----------------------------------------------------------------------------
_CACHED = {}


def kernel(**inputs):
    feats = np.asarray(inputs["feats"], dtype=np.float32)
    s = np.asarray(inputs["s"], dtype=np.float32)
    src = np.asarray(inputs["src"])
    dst = np.asarray(inputs["dst"])

    in_maps, meta = _preprocess(feats, s, src, dst)

    from concourse.bass_utils import run_bass_kernel_spmd

    nc = _CACHED.get("nc")
    if nc is None:
        nc = _build()
        _CACHED["nc"] = nc

    res = run_bass_kernel_spmd(nc, in_maps, core_ids=list(range(N_CORES)))
    _CACHED["last_result"] = res
    out = np.zeros((N_NODES, D), dtype=np.float32)
    core_of, wloc_of, pos_of = meta["core_of"], meta["wloc_of"], meta["pos_of"]
    rows = pos_of.astype(np.int64) * W_PER_CORE + wloc_of.astype(np.int64)
    for c in range(N_CORES):
        m = core_of == c
        out[m] = res.results[c]["out_pm"][rows[m]]
    return out


if __name__ == "__main__":
    nc = _build()
    print("build ok")
